# revision 2
# baseline (speedup 1.0000x reference)
"""Trainium2 Bass kernel for nn_ClusterLoss_Regr (topk_masking).

Computes  mean_b(128 - max_p((128 - d[b,p]) * [|proto[p] - label[b]| <= 0.5]))
for d: [8192, 4096] f32, labels: [8192] f32, proto: [4096] f32 -> scalar f32.

Sharding: data-parallel over the batch axis across 8 NeuronCores (1024 rows
per core); proto_classes replicated; final mean on host.

Device schedule (per core; memory-bound, 16 MiB HBM read):
  - Host appends each row's label as column 4096 of that row, so every
    [128, 4097] row-tile DMA carries its labels inside the same 16 KiB
    full-row descriptors (no separate labels DMA; a tiny-descriptor labels
    transfer would stall the single HWDGE descriptor generator).
  - All 8 row-tile DMAs are full tiles on the sync HWDGE ring, 16 KiB
    descriptors only: the generator (~30 ns/desc) keeps ahead of the
    ~434 GB/s drain. (Splitting the last tile into column-quarters makes
    4 KiB descriptors, which are generator-bound and stall the tail.)
  - proto row rides first on the ring, then GPSIMD broadcasts it to all
    128 partitions in two column-halves so the first DVE op can start
    before the full broadcast finishes.
  - One fused custom-DVE op per tile:
        out[p,k] = select((proto[k] - label[p])^2 <= 0.25, 128 - d[p,k], 0)
        accum[p] = max_k out[p,k]
    bit-exact with the reference mask/multiply/top_k(1) chain (f32 monotone
    rounding: |x| <= 0.5 <=> fl(x^2) <= 0.25). Tile 0 runs as two
    column-half ops gated on the two broadcast halves.
Host: gather [8192] row maxima, loss = mean(128 - rowmax) (f64 accumulate,
cast to f32).
"""

import numpy as np

B, P = 8192, 4096
NCORES = 8
BSH = B // NCORES  # 1024 rows per core
RT = BSH // 128    # 8 row-tiles of 128 rows
PW = P + 1         # row width incl. label column
MAX_DIST = np.float32(128.0)
NCOLS = RT + 1     # stats columns: 0,1 = tile0 halves; 2..8 = tiles 1..7

_cache: dict = {}


def _ensure_path():
    try:
        import concourse.bass  # noqa: F401
    except ImportError:
        import sys

        for p in ("/opt/trn_rl_repo",):
            if p not in sys.path:
                sys.path.insert(0, p)


def _register_dve_op():
    """Register the fused mask+invert+rowmax op in the custom-DVE registry.

    Idempotent; computes its own uops_sha so no golden file is needed.
    """
    from concourse import dve_ops
    from concourse.dve_spec import (
        C0,
        C1,
        C2,
        Spec,
        Src0,
        Src1,
        Zero,
        lower,
        maxx,
        select,
        sq,
    )
    from concourse.dve_uop import DveOpSpec

    name = "CLUSTER_MASK_MAX_ANT"
    for op in dve_ops.OPS:
        if op.name == name:
            return op

    def _ref(in0, in1, s0, s1, imm2):
        o = np.where(
            (in1.astype(np.float32) - s0) ** 2 <= imm2,
            (np.float32(s1) - in0).astype(np.float32),
            np.float32(0.0),
        ).astype(np.float32)
        return o, o.max(axis=-1, keepdims=True)

    spec = Spec(
        body=select(sq(Src1 - C0) <= C2, C1 - Src0, Zero),
        accum=maxx,
        accum_init=Zero,
        reference=_ref,
    )
    shas: dict = {}
    op = dve_ops.DveOp(name, spec, subdim=False, uops_sha=shas)
    dve_ops.OPS.append(op)
    row = dve_ops._CUSTOM_DVE_ROW_BASE + len(dve_ops.OPS) - 1
    dve_ops._SUB_OPCODE_FOR_NAME[name] = row
    dve_ops.CUSTOM_DVE_SPECS[name] = spec
    for ver in ("v3", "v4"):
        shas[ver] = DveOpSpec(
            name=name, opcode=row, uops=lower(spec, ver=ver), rd1_en=True
        ).sha(ver)
    return op


def _get_bass():
    if "nc" in _cache:
        return _cache["nc"]
    _ensure_path()
    import concourse.bacc as bacc
    import concourse.mybir as mybir

    op = _register_dve_op()
    f32 = mybir.dt.float32
    nc = bacc.Bacc(
        "TRN2", target_bir_lowering=False, debug=False, num_devices=NCORES
    )
    # dx row r = [d[r, 0:4096], label[r]]  (4097 f32 = one 16388 B descriptor
    # per partition line -> labels arrive with their tile's DMA)
    dx_ap = nc.dram_tensor("dx", [BSH, PW], f32, kind="ExternalInput").ap()
    proto_ap = nc.dram_tensor("proto", [P], f32, kind="ExternalInput").ap()
    out_ap = nc.dram_tensor("rowmax", [128, NCOLS], f32, kind="ExternalOutput").ap()

    prow = nc.alloc_sbuf_tensor("prow", [1, P], f32).ap()
    proto_tile = nc.alloc_sbuf_tensor("proto_tile", [128, P], f32).ap()
    stats = nc.alloc_sbuf_tensor("stats", [128, NCOLS], f32).ap()
    scratch = nc.alloc_sbuf_tensor("scratch", [128, P], f32).ap()
    d_tiles = [nc.alloc_sbuf_tensor(f"dt{t}", [128, PW], f32).ap() for t in range(RT)]

    H = P // 2  # broadcast / tile-0 column-half width

    # One dedicated semaphore per DMA (a DMA's sem is bumped +1 by each of
    # the 16 SDMA engines; a dedicated sem == 16 is exact).
    d_sems = [nc.alloc_semaphore(f"d_sem{t}") for t in range(RT)]
    prow_sem = nc.alloc_semaphore("prow_sem")
    out_sem = nc.alloc_semaphore("out_sem")
    pb_sem = nc.alloc_semaphore("pb_sem")
    dve_sem = nc.alloc_semaphore("dve_sem")

    NV = RT + 1  # DVE ops: 2 halves of tile 0 + tiles 1..7

    with nc.Block() as block:

        @block.sync
        def _(sync):
            # proto first: tiny, lands immediately, unblocks the broadcast.
            sync.dma_start(prow[:], proto_ap[None, :]).then_inc(prow_sem, 16)
            for t in range(RT):
                sync.dma_start(
                    d_tiles[t][:],
                    dx_ap[128 * t : 128 * (t + 1), :],
                ).then_inc(d_sems[t], 16)
            sync.wait_ge(dve_sem, NV)
            sync.dma_start(out_ap[:], stats[:]).then_inc(out_sem, 16)
            sync.wait_ge(out_sem, 16)
            # Reset all kernel semaphores so re-executing the loaded NEFF
            # behaves identically to the first run.
            all_sems = sorted(
                s.num for s in [*d_sems, prow_sem, out_sem, pb_sem, dve_sem]
            )
            lo = prev = all_sems[0]
            for n in all_sems[1:] + [None]:
                if n is not None and n == prev + 1:
                    prev = n
                    continue
                sync.sem_clear(range(lo, prev + 1))
                if n is not None:
                    lo = prev = n

        @block.gpsimd
        def _(gpsimd):
            gpsimd.wait_ge(prow_sem, 16)
            gpsimd.partition_broadcast(
                proto_tile[:, 0:H], prow[:, 0:H]
            ).then_inc(pb_sem, 1)
            gpsimd.partition_broadcast(
                proto_tile[:, H:P], prow[:, H:P]
            ).then_inc(pb_sem, 1)

        @block.vector
        def _(vector):
            def dve(t, lo, w, col):
                return nc.vector._custom_dve(
                    op,
                    out=scratch[:, lo : lo + w],
                    in0=d_tiles[t][:, lo : lo + w],
                    in1=proto_tile[:, lo : lo + w],
                    s0=d_tiles[t][:, P : P + 1],
                    s1=float(MAX_DIST),
                    imm2=0.25,
                    accum_out=stats[:, col : col + 1],
                )

            vector.wait_ge(pb_sem, 1)
            vector.wait_ge(d_sems[0], 16)
            dve(0, 0, H, 0).then_inc(dve_sem, 1)
            vector.wait_ge(pb_sem, 2)
            dve(0, H, H, 1).then_inc(dve_sem, 1)
            for t in range(1, RT):
                vector.wait_ge(d_sems[t], 16)
                dve(t, 0, P, t + 1).then_inc(dve_sem, 1)

    nc.compile()
    _cache["nc"] = nc
    return nc


def _run_device(min_distances, labels, proto_classes, trace=False):
    nc = _get_bass()
    from concourse.bass_utils import run_bass_kernel_spmd

    proto = np.ascontiguousarray(np.asarray(proto_classes, dtype=np.float32))
    md = np.asarray(min_distances, dtype=np.float32)
    lab = np.asarray(labels, dtype=np.float32)
    in_maps = []
    for c in range(NCORES):
        dx = np.empty((BSH, PW), dtype=np.float32)
        dx[:, :P] = md[c * BSH : (c + 1) * BSH]
        dx[:, P] = lab[c * BSH : (c + 1) * BSH]
        in_maps.append({"dx": dx, "proto": proto})
    return run_bass_kernel_spmd(
        nc, in_maps, core_ids=list(range(NCORES)), trace=trace
    )


def kernel(min_distances, labels, proto_classes):
    res = _run_device(min_distances, labels, proto_classes).results
    # stats columns: 0,1 = halves of tile 0 (combine by max); 2..8 = tiles
    # 1..7. Row = 1024*c + 128*t + p.
    stats = np.stack([np.asarray(res[c]["rowmax"]) for c in range(NCORES)])
    t0 = np.maximum(stats[:, :, 0], stats[:, :, 1])
    rowmax = np.concatenate([t0[:, :, None], stats[:, :, 2:]], axis=2)
    rowmax = rowmax.transpose(0, 2, 1).reshape(B).astype(np.float32)
    loss_rows = (MAX_DIST - rowmax).astype(np.float32)
    return np.array(loss_rows.mean(dtype=np.float64), dtype=np.float32)


# revision 7
# speedup vs baseline: 1.0897x; 1.0897x over previous
"""Trainium2 Bass kernel for nn_ClusterLoss_Regr (topk_masking).

Computes  mean_b(128 - max_p((128 - d[b,p]) * [|proto[p] - label[b]| <= 0.5]))
for d: [8192, 4096] f32, labels: [8192] f32, proto: [4096] f32 -> scalar f32.

Sharding: data-parallel over the batch axis across 8 NeuronCores (1024 rows
per core); proto_classes replicated; final mean on host.

Device schedule (per core, fp16 stream):
  - Host casts the stream to fp16 (tolerance is 2e-2; fp16 end-to-end error
    is ~4e-5). Labels stay f32 (the DVE scalar port requires f32) and load
    as a tiny [128, 8] tensor via GPSIMD/SWDGE so their 128 tiny
    descriptors never touch the HWDGE ring's generator.
  - 8 full row-tile DMAs on the sync HWDGE ring (8192 B descriptors). This
    halves HBM bytes (8.4 MiB/core), which also drops per-core demand below
    the fair share of the ~716 GB/s HBM stack that each NeuronCore PAIR
    shares - in f32 the pair contends and the losing core's stream
    stretches, inflating the max-core time.
  - proto rides first on the ring; GPSIMD broadcasts it to 128 partitions
    in two column-halves, off the SDMA path.
  - One fused custom-DVE op per tile:
        out[p,k] = select((proto[k] - label[p])^2 <= 0.25, 128 - d[p,k], 0)
        accum[p] = max_k out[p,k]
    (f32 ALU on fp16-rounded inputs; monotone, so the mask/top-1 structure
    is preserved exactly on the rounded values).
  - The stats write-back ([128,8] f32 -> 144 tiny descriptors) is issued on
    GPSIMD/SWDGE: Q7 generates descriptors ~4x faster than the HWDGE ring,
    keeping the tail off the hardware generator's critical path.
Host: gather [8192] row maxima, loss = mean(128 - rowmax) (f64 accumulate,
cast to f32).
"""

import numpy as np

B, P = 8192, 4096
NCORES = 8
BSH = B // NCORES  # 1024 rows per core
RT = BSH // 128    # 8 row-tiles of 128 rows
PW = P             # row width (pure d; labels load separately)
MAX_DIST = np.float32(128.0)
NCOLS = RT

_cache: dict = {}


def _ensure_path():
    try:
        import concourse.bass  # noqa: F401
    except ImportError:
        import sys

        for p in ("/opt/trn_rl_repo",):
            if p not in sys.path:
                sys.path.insert(0, p)


def _register_dve_op():
    """Register the fused mask+invert+rowmax op in the custom-DVE registry.

    Idempotent; computes its own uops_sha so no golden file is needed.
    """
    from concourse import dve_ops
    from concourse.dve_spec import (
        C0,
        C1,
        C2,
        Spec,
        Src0,
        Src1,
        Zero,
        lower,
        maxx,
        select,
        sq,
    )
    from concourse.dve_uop import DveOpSpec

    name = "CLUSTER_MASK_MAX_ANT"
    for op in dve_ops.OPS:
        if op.name == name:
            return op

    def _ref(in0, in1, s0, s1, imm2):
        o = np.where(
            (in1.astype(np.float32) - np.float32(s0)) ** 2 <= imm2,
            (np.float32(s1) - in0.astype(np.float32)).astype(np.float32),
            np.float32(0.0),
        ).astype(np.float32)
        return o, o.max(axis=-1, keepdims=True)

    spec = Spec(
        body=select(sq(Src1 - C0) <= C2, C1 - Src0, Zero),
        accum=maxx,
        accum_init=Zero,
        reference=_ref,
    )
    shas: dict = {}
    op = dve_ops.DveOp(name, spec, subdim=False, uops_sha=shas)
    dve_ops.OPS.append(op)
    row = dve_ops._CUSTOM_DVE_ROW_BASE + len(dve_ops.OPS) - 1
    dve_ops._SUB_OPCODE_FOR_NAME[name] = row
    dve_ops.CUSTOM_DVE_SPECS[name] = spec
    for ver in ("v3", "v4"):
        shas[ver] = DveOpSpec(
            name=name, opcode=row, uops=lower(spec, ver=ver), rd1_en=True
        ).sha(ver)
    return op


def _get_bass():
    if "nc" in _cache:
        return _cache["nc"]
    _ensure_path()
    import concourse.bacc as bacc
    import concourse.mybir as mybir

    op = _register_dve_op()
    f16 = mybir.dt.float16
    f32 = mybir.dt.float32
    nc = bacc.Bacc(
        "TRN2", target_bir_lowering=False, debug=False, num_devices=NCORES
    )
    dx_ap = nc.dram_tensor("dx", [BSH, PW], f16, kind="ExternalInput").ap()
    lab_ap = nc.dram_tensor("labels_col", [128, RT], f32, kind="ExternalInput").ap()
    proto_ap = nc.dram_tensor("proto", [P], f16, kind="ExternalInput").ap()
    out_ap = nc.dram_tensor("rowmax", [128, NCOLS], f32, kind="ExternalOutput").ap()

    prow = nc.alloc_sbuf_tensor("prow", [1, P], f16).ap()
    proto_tile = nc.alloc_sbuf_tensor("proto_tile", [128, P], f16).ap()
    stats = nc.alloc_sbuf_tensor("stats", [128, NCOLS], f32).ap()
    labels_tile = nc.alloc_sbuf_tensor("labels_tile", [128, RT], f32).ap()
    scratch = nc.alloc_sbuf_tensor("scratch", [128, P], f16).ap()
    d_tiles = [nc.alloc_sbuf_tensor(f"dt{t}", [128, PW], f16).ap() for t in range(RT)]

    H = P // 2  # broadcast column-half width

    # One dedicated semaphore per DMA (a DMA's sem is bumped +1 by each of
    # the 16 SDMA engines; a dedicated sem == 16 is exact).
    d_sems = [nc.alloc_semaphore(f"d_sem{t}") for t in range(RT)]
    prow_sem = nc.alloc_semaphore("prow_sem")
    lab_sem = nc.alloc_semaphore("lab_sem")
    out_sem = nc.alloc_semaphore("out_sem")
    pb_sem = nc.alloc_semaphore("pb_sem")
    dve_sem = nc.alloc_semaphore("dve_sem")

    NV = RT  # DVE ops

    with nc.Block() as block:

        @block.sync
        def _(sync):
            # proto first: tiny, lands immediately, unblocks the broadcast.
            sync.dma_start(prow[:], proto_ap[None, :]).then_inc(prow_sem, 16)
            for t in range(RT):
                sync.dma_start(
                    d_tiles[t][:],
                    dx_ap[128 * t : 128 * (t + 1), :],
                ).then_inc(d_sems[t], 16)
            sync.wait_ge(out_sem, 16)
            # Reset all kernel semaphores so re-executing the loaded NEFF
            # behaves identically to the first run.
            all_sems = sorted(
                s.num
                for s in [*d_sems, prow_sem, lab_sem, out_sem, pb_sem, dve_sem]
            )
            lo = prev = all_sems[0]
            for n in all_sems[1:] + [None]:
                if n is not None and n == prev + 1:
                    prev = n
                    continue
                sync.sem_clear(range(lo, prev + 1))
                if n is not None:
                    lo = prev = n

        @block.scalar
        def _(scalar):
            scalar.dma_start(labels_tile[:], lab_ap[:]).then_inc(lab_sem, 16)
            scalar.wait_ge(dve_sem, NV)
            scalar.dma_start(out_ap[:], stats[:]).then_inc(out_sem, 16)

        @block.gpsimd
        def _(gpsimd):
            gpsimd.wait_ge(prow_sem, 16)
            gpsimd.partition_broadcast(
                proto_tile[:, 0:H], prow[:, 0:H]
            ).then_inc(pb_sem, 1)
            gpsimd.partition_broadcast(
                proto_tile[:, H:P], prow[:, H:P]
            ).then_inc(pb_sem, 1)

        @block.vector
        def _(vector):
            vector.wait_ge(pb_sem, 2)
            vector.wait_ge(lab_sem, 16)
            for t in range(RT):
                vector.wait_ge(d_sems[t], 16)
                if t:
                    # Trivially-true edge: orders scratch reuse for the race
                    # detector (the DVE serializes its own ops on hardware).
                    vector.wait_ge(dve_sem, t)
                nc.vector._custom_dve(
                    op,
                    out=scratch[:],
                    in0=d_tiles[t][:, 0:P],
                    in1=proto_tile[:],
                    s0=labels_tile[:, t : t + 1],
                    s1=float(MAX_DIST),
                    imm2=0.25,
                    accum_out=stats[:, t : t + 1],
                ).then_inc(dve_sem, 1)

    nc.compile()
    _cache["nc"] = nc
    return nc


def _run_device(min_distances, labels, proto_classes, trace=False):
    nc = _get_bass()
    from concourse.bass_utils import run_bass_kernel_spmd

    proto = np.ascontiguousarray(np.asarray(proto_classes, dtype=np.float16))
    md = np.asarray(min_distances, dtype=np.float32)
    lab = np.asarray(labels, dtype=np.float32)
    in_maps = []
    for c in range(NCORES):
        dx = np.ascontiguousarray(
            md[c * BSH : (c + 1) * BSH].astype(np.float16)
        )
        lsh = np.ascontiguousarray(
            lab[c * BSH : (c + 1) * BSH].reshape(RT, 128).T
        )
        in_maps.append({"dx": dx, "labels_col": lsh, "proto": proto})
    return run_bass_kernel_spmd(
        nc, in_maps, core_ids=list(range(NCORES)), trace=trace
    )


def kernel(min_distances, labels, proto_classes):
    res = _run_device(min_distances, labels, proto_classes).results
    # stats column t = tile t. Row = 1024*c + 128*t + p.
    stats = np.stack([np.asarray(res[c]["rowmax"]) for c in range(NCORES)])
    rowmax = stats.transpose(0, 2, 1).reshape(B).astype(np.float32)
    loss_rows = (MAX_DIST - rowmax).astype(np.float32)
    return np.array(loss_rows.mean(dtype=np.float64), dtype=np.float32)


# revision 9
# speedup vs baseline: 1.2757x; 1.1707x over previous
"""Trainium2 Bass kernel for nn_ClusterLoss_Regr (topk_masking).

Computes  mean_b(128 - max_p((128 - d[b,p]) * [|proto[p] - label[b]| <= 0.5]))
for d: [8192, 4096] f32, labels: [8192] f32, proto: [4096] f32 -> scalar f32.

Sharding: data-parallel over the batch axis across 8 NeuronCores (1024 rows
per core); proto_classes replicated; final mean on host.

Device schedule (per core, fp16 stream):
  - Host casts the stream to fp16 (tolerance is 2e-2; fp16 end-to-end error
    is ~4e-5). Labels stay f32 (the DVE scalar port requires f32) and load
    as a tiny [128, 8] tensor via GPSIMD/SWDGE so their 128 tiny
    descriptors never touch the HWDGE ring's generator.
  - 8 full row-tile DMAs on the sync HWDGE ring (8192 B descriptors). This
    halves HBM bytes (8.4 MiB/core), which also drops per-core demand below
    the fair share of the ~716 GB/s HBM stack that each NeuronCore PAIR
    shares - in f32 the pair contends and the losing core's stream
    stretches, inflating the max-core time.
  - proto arrives host-replicated to [128, 4096] fp16 (1 MiB, +12% bytes)
    at the head of the sync ring: a plain DMA replaces the GPSIMD
    partition_broadcast, whose first instruction paid a ~10 us Q7 warm-up
    that delayed the whole DVE chain by ~12 us. GPSIMD is entirely unused.
  - One fused custom-DVE op per tile:
        out[p,k] = select((proto[k] - label[p])^2 <= 0.25, 128 - d[p,k], 0)
        accum[p] = max_k out[p,k]
    (f32 ALU on fp16-rounded inputs; monotone, so the mask/top-1 structure
    is preserved exactly on the rounded values).
  - The stats write-back ([128,8] f32 -> 144 tiny descriptors) is issued on
    GPSIMD/SWDGE: Q7 generates descriptors ~4x faster than the HWDGE ring,
    keeping the tail off the hardware generator's critical path.
Host: gather [8192] row maxima, loss = mean(128 - rowmax) (f64 accumulate,
cast to f32).
"""

import numpy as np

B, P = 8192, 4096
NCORES = 8
BSH = B // NCORES  # 1024 rows per core
RT = BSH // 128    # 8 row-tiles of 128 rows
PW = P             # row width (pure d; labels load separately)
MAX_DIST = np.float32(128.0)
NCOLS = RT

_cache: dict = {}


def _ensure_path():
    try:
        import concourse.bass  # noqa: F401
    except ImportError:
        import sys

        for p in ("/opt/trn_rl_repo",):
            if p not in sys.path:
                sys.path.insert(0, p)


def _register_dve_op():
    """Register the fused mask+invert+rowmax op in the custom-DVE registry.

    Idempotent; computes its own uops_sha so no golden file is needed.
    """
    from concourse import dve_ops
    from concourse.dve_spec import (
        C0,
        C1,
        C2,
        Spec,
        Src0,
        Src1,
        Zero,
        lower,
        maxx,
        select,
        sq,
    )
    from concourse.dve_uop import DveOpSpec

    name = "CLUSTER_MASK_MAX_ANT"
    for op in dve_ops.OPS:
        if op.name == name:
            return op

    def _ref(in0, in1, s0, s1, imm2):
        o = np.where(
            (in1.astype(np.float32) - np.float32(s0)) ** 2 <= imm2,
            (np.float32(s1) - in0.astype(np.float32)).astype(np.float32),
            np.float32(0.0),
        ).astype(np.float32)
        return o, o.max(axis=-1, keepdims=True)

    spec = Spec(
        body=select(sq(Src1 - C0) <= C2, C1 - Src0, Zero),
        accum=maxx,
        accum_init=Zero,
        reference=_ref,
    )
    shas: dict = {}
    op = dve_ops.DveOp(name, spec, subdim=False, uops_sha=shas)
    dve_ops.OPS.append(op)
    row = dve_ops._CUSTOM_DVE_ROW_BASE + len(dve_ops.OPS) - 1
    dve_ops._SUB_OPCODE_FOR_NAME[name] = row
    dve_ops.CUSTOM_DVE_SPECS[name] = spec
    for ver in ("v3", "v4"):
        shas[ver] = DveOpSpec(
            name=name, opcode=row, uops=lower(spec, ver=ver), rd1_en=True
        ).sha(ver)
    return op


def _get_bass():
    if "nc" in _cache:
        return _cache["nc"]
    _ensure_path()
    import concourse.bacc as bacc
    import concourse.mybir as mybir

    op = _register_dve_op()
    f16 = mybir.dt.float16
    f32 = mybir.dt.float32
    nc = bacc.Bacc(
        "TRN2", target_bir_lowering=False, debug=False, num_devices=NCORES
    )
    dx_ap = nc.dram_tensor("dx", [BSH, PW], f16, kind="ExternalInput").ap()
    lab_ap = nc.dram_tensor("labels_col", [128, RT], f32, kind="ExternalInput").ap()
    proto_ap = nc.dram_tensor("proto_rep", [128, P], f16, kind="ExternalInput").ap()
    out_ap = nc.dram_tensor("rowmax", [128, NCOLS], f32, kind="ExternalOutput").ap()

    proto_tile = nc.alloc_sbuf_tensor("proto_tile", [128, P], f16).ap()
    stats = nc.alloc_sbuf_tensor("stats", [128, NCOLS], f32).ap()
    labels_tile = nc.alloc_sbuf_tensor("labels_tile", [128, RT], f32).ap()
    scratch = nc.alloc_sbuf_tensor("scratch", [128, P], f16).ap()
    d_tiles = [nc.alloc_sbuf_tensor(f"dt{t}", [128, PW], f16).ap() for t in range(RT)]

    # One dedicated semaphore per DMA (a DMA's sem is bumped +1 by each of
    # the 16 SDMA engines; a dedicated sem == 16 is exact).
    d_sems = [nc.alloc_semaphore(f"d_sem{t}") for t in range(RT)]
    proto_sem = nc.alloc_semaphore("proto_sem")
    lab_sem = nc.alloc_semaphore("lab_sem")
    out_sem = nc.alloc_semaphore("out_sem")
    dve_sem = nc.alloc_semaphore("dve_sem")

    NV = RT  # DVE ops

    with nc.Block() as block:

        @block.sync
        def _(sync):
            # proto first: lands before tile 0 so the DVE can start at once.
            sync.dma_start(proto_tile[:], proto_ap[:]).then_inc(proto_sem, 16)
            for t in range(RT):
                sync.dma_start(
                    d_tiles[t][:],
                    dx_ap[128 * t : 128 * (t + 1), :],
                ).then_inc(d_sems[t], 16)
            sync.wait_ge(out_sem, 16)
            # Reset all kernel semaphores so re-executing the loaded NEFF
            # behaves identically to the first run.
            all_sems = sorted(
                s.num
                for s in [*d_sems, proto_sem, lab_sem, out_sem, dve_sem]
            )
            lo = prev = all_sems[0]
            for n in all_sems[1:] + [None]:
                if n is not None and n == prev + 1:
                    prev = n
                    continue
                sync.sem_clear(range(lo, prev + 1))
                if n is not None:
                    lo = prev = n

        @block.scalar
        def _(scalar):
            scalar.dma_start(labels_tile[:], lab_ap[:]).then_inc(lab_sem, 16)
            scalar.wait_ge(dve_sem, NV)
            scalar.dma_start(out_ap[:], stats[:]).then_inc(out_sem, 16)

        @block.vector
        def _(vector):
            vector.wait_ge(proto_sem, 16)
            vector.wait_ge(lab_sem, 16)
            for t in range(RT):
                vector.wait_ge(d_sems[t], 16)
                if t:
                    # Trivially-true edge: orders scratch reuse for the race
                    # detector (the DVE serializes its own ops on hardware).
                    vector.wait_ge(dve_sem, t)
                nc.vector._custom_dve(
                    op,
                    out=scratch[:],
                    in0=d_tiles[t][:, 0:P],
                    in1=proto_tile[:],
                    s0=labels_tile[:, t : t + 1],
                    s1=float(MAX_DIST),
                    imm2=0.25,
                    accum_out=stats[:, t : t + 1],
                ).then_inc(dve_sem, 1)

    nc.compile()
    _cache["nc"] = nc
    return nc


def _run_device(min_distances, labels, proto_classes, trace=False):
    nc = _get_bass()
    from concourse.bass_utils import run_bass_kernel_spmd

    proto = np.ascontiguousarray(
        np.broadcast_to(
            np.asarray(proto_classes, dtype=np.float16)[None, :], (128, P)
        )
    )
    md = np.asarray(min_distances, dtype=np.float32)
    lab = np.asarray(labels, dtype=np.float32)
    in_maps = []
    for c in range(NCORES):
        dx = np.ascontiguousarray(
            md[c * BSH : (c + 1) * BSH].astype(np.float16)
        )
        lsh = np.ascontiguousarray(
            lab[c * BSH : (c + 1) * BSH].reshape(RT, 128).T
        )
        in_maps.append({"dx": dx, "labels_col": lsh, "proto_rep": proto})
    return run_bass_kernel_spmd(
        nc, in_maps, core_ids=list(range(NCORES)), trace=trace
    )


def kernel(min_distances, labels, proto_classes):
    res = _run_device(min_distances, labels, proto_classes).results
    # stats column t = tile t. Row = 1024*c + 128*t + p.
    stats = np.stack([np.asarray(res[c]["rowmax"]) for c in range(NCORES)])
    rowmax = stats.transpose(0, 2, 1).reshape(B).astype(np.float32)
    loss_rows = (MAX_DIST - rowmax).astype(np.float32)
    return np.array(loss_rows.mean(dtype=np.float64), dtype=np.float32)
